# revision 1
# baseline (speedup 1.0000x reference)
"""Multi-head self-attention with RoPE on 8 Trainium2 NeuronCores.

Sharding: tensor-parallel over heads (16 heads / 8 cores = 2 heads per core).
Each core computes Q/K/V projections for its 2 heads over all 4 batches,
causal flash-style attention (scores computed transposed, no on-chip
transposes needed), and a partial output projection over its 128 columns of
Wo's input dim. Host sums the 8 partial outputs.

Self-contained: hardcodes all shapes from the problem spec.
"""

import numpy as np
import ml_dtypes

BF16 = ml_dtypes.bfloat16

B, S, DM = 4, 2048, 1024
H, DH = 16, 64
NCORES = 8
HPC = H // NCORES  # 2 heads per core
DL = HPC * DH  # 128 local head dims per core
SB = 512  # q-block / s-block width
NSB = S // SB  # 4
NKT = S // 128  # 16 k-tiles per batch
LN_THETA = float(np.log(10000.0))

# tunables (PSUM bank budget: S_BUFS*(2 if SMODE=="pair" else 1) + Y_BUFS + PS_BUFS <= 8)
CFG = {
    "SMODE": "pair",
    "S_BUFS": 2,
    "Y_BUFS": 2,
    "PS_BUFS": 1,
    "DEFER": True,
    "LB_IN_Y": False,
    "PROJ_BUFS": 1,
    "QK_BUFS": 3,
    "XT_BUFS": 8,
    "V_BUFS": 3,
    "L_BUFS": 3,
    "P_BUFS": 4,
}


def _build_nc(reps=1):
    import concourse.bass as bass
    import concourse.tile as tile
    import concourse.mybir as mybir
    from concourse import bacc

    dt = mybir.dt
    F32 = dt.float32
    BF = dt.bfloat16
    AF = mybir.ActivationFunctionType

    nc = bacc.Bacc("TRN2", target_bir_lowering=False, debug=False)

    xt_d = nc.dram_tensor("xt", [B, DM, S], BF, kind="ExternalInput").ap()
    wq_d = nc.dram_tensor("wqt", [DM, DL], BF, kind="ExternalInput").ap()
    wk_d = nc.dram_tensor("wkt", [DM, DL], BF, kind="ExternalInput").ap()
    wv_d = nc.dram_tensor("wvt", [DM, DL], BF, kind="ExternalInput").ap()
    wo_d = nc.dram_tensor("wot", [DL, DM], BF, kind="ExternalInput").ap()
    pos_d = nc.dram_tensor("pos", [1, S], dt.int32, kind="ExternalInput").ap()
    out_d = nc.dram_tensor("outp", [B, S, DM], F32, kind="ExternalOutput").ap()

    with tile.TileContext(nc) as tc:
        import contextlib

        ctx = contextlib.ExitStack()
        with ctx:
            # ---------------- pools ----------------
            consts = ctx.enter_context(tc.tile_pool(name="consts", bufs=1))
            xt_p = ctx.enter_context(tc.tile_pool(name="xt", bufs=CFG.get("XT_BUFS", 12)))
            qk_p = ctx.enter_context(tc.tile_pool(name="qk", bufs=CFG.get("QK_BUFS", 2)))
            rope_p = ctx.enter_context(tc.tile_pool(name="rope", bufs=CFG.get("R_BUFS", 2)))
            v_p = ctx.enter_context(tc.tile_pool(name="v", bufs=CFG.get("V_BUFS", 2)))
            p_p = ctx.enter_context(tc.tile_pool(name="p", bufs=CFG.get("P_BUFS", 3)))
            ysb_p = ctx.enter_context(tc.tile_pool(name="ysb", bufs=CFG.get("YS_BUFS", 3)))
            lin_p = ctx.enter_context(tc.tile_pool(name="lin", bufs=CFG.get("L_BUFS", 2)))
            outs_p = ctx.enter_context(tc.tile_pool(name="outs", bufs=CFG.get("O_BUFS", 4)))
            ps_p = ctx.enter_context(
                tc.tile_pool(name="ps", bufs=CFG["PS_BUFS"], space="PSUM")
            )
            if CFG.get("PROJ_BUFS", 1) > 0:
                ps2_p = ctx.enter_context(
                    tc.tile_pool(name="ps2", bufs=CFG.get("PROJ_BUFS", 1), space="PSUM")
                )
            else:
                ps2_p = ps_p
            s_p = ctx.enter_context(
                tc.tile_pool(name="s", bufs=CFG["S_BUFS"], space="PSUM")
            )
            y_p = ctx.enter_context(
                tc.tile_pool(name="y", bufs=CFG["Y_BUFS"], space="PSUM")
            )

            # ---------------- weights -> SBUF ----------------
            wq_sb = consts.tile([128, 8, DL], BF, tag="wq")
            wk_sb = consts.tile([128, 8, DL], BF, tag="wk")
            wv_sb = consts.tile([128, 8, DL], BF, tag="wv")
            wo_sb = consts.tile([128, DM], BF, tag="wo")
            nc.sync.dma_start(out=wq_sb, in_=wq_d.rearrange("(t p) d -> p t d", p=128))
            nc.sync.dma_start(out=wk_sb, in_=wk_d.rearrange("(t p) d -> p t d", p=128))
            nc.sync.dma_start(out=wv_sb, in_=wv_d.rearrange("(t p) d -> p t d", p=128))
            nc.sync.dma_start(out=wo_sb, in_=wo_d)

            # ---------------- cos/sin tables ----------------
            # invf row [1, 32]: exp(-j * 2*ln(theta)/64)
            invf_i = consts.tile([1, 32], dt.int32, tag="invf_i")
            nc.gpsimd.iota(invf_i, pattern=[[1, 32]], base=0, channel_multiplier=0)
            invf_f = consts.tile([1, 32], F32, tag="invf_f")
            nc.vector.tensor_copy(invf_f, invf_i)
            invf = consts.tile([1, 32], F32, tag="invf")
            nc.scalar.activation(invf, invf_f, AF.Exp, scale=-(2.0 * LN_THETA / 64.0))

            pos_i = consts.tile([1, S], dt.int32, tag="pos_i")
            nc.sync.dma_start(out=pos_i, in_=pos_d)
            pos_f = consts.tile([1, S], F32, tag="pos_f")
            nc.vector.tensor_copy(pos_f, pos_i)

            sin32 = consts.tile([32, S], BF, tag="sin32")
            nsin32 = consts.tile([32, S], BF, tag="nsin32")
            cos32 = consts.tile([32, S], BF, tag="cos32")
            # Sin LUT needs args in [-pi, pi]: Cody-Waite range reduction.
            # HW f32->i32 conversion rounds to nearest; CoreSim truncates.
            # The is_gt fix-up makes the result exact under both (args >= 0).
            INV2PI = float(1.0 / (2.0 * np.pi))
            C1 = 6.28125
            C2 = float(2.0 * np.pi - 6.28125)
            TWO_PI = float(2.0 * np.pi)

            def reduce_to_pi(x):
                # x >= 0 (SBUF or PSUM AP) -> SBUF f32 in [-pi, pi]
                t = rope_p.tile([32, SB], F32, tag="rr_t")
                nc.vector.tensor_scalar_mul(t, x, INV2PI)
                ri = rope_p.tile([32, SB], dt.int32, tag="rr_i")
                nc.vector.tensor_copy(ri, t)
                rf = rope_p.tile([32, SB], F32, tag="rr_f")
                nc.vector.tensor_copy(rf, ri)
                a1 = rope_p.tile([32, SB], F32, tag="rr_a1")
                nc.vector.scalar_tensor_tensor(
                    a1, rf, -C1, x,
                    op0=mybir.AluOpType.mult, op1=mybir.AluOpType.add,
                )
                a2 = rope_p.tile([32, SB], F32, tag="rr_a2")
                nc.vector.scalar_tensor_tensor(
                    a2, rf, -C2, a1,
                    op0=mybir.AluOpType.mult, op1=mybir.AluOpType.add,
                )
                over = rope_p.tile([32, SB], F32, tag="rr_ov")
                nc.vector.tensor_scalar(
                    over, a2, float(np.pi), None, op0=mybir.AluOpType.is_gt
                )
                a3 = rope_p.tile([32, SB], F32, tag="rr_a3")
                nc.vector.scalar_tensor_tensor(
                    a3, over, -TWO_PI, a2,
                    op0=mybir.AluOpType.mult, op1=mybir.AluOpType.add,
                )
                return a3

            for cchunk in range(NSB):
                csl = slice(cchunk * SB, (cchunk + 1) * SB)
                ang = ps_p.tile([128, SB], F32, tag="ps1")
                # angles = outer(invf, pos) via K=1 fp32 matmul
                nc.tensor.matmul(
                    ang[0:32, :], lhsT=invf, rhs=pos_f[:, csl], start=True, stop=True
                )
                angv = ang[0:32, :]
                a_s = reduce_to_pi(angv)
                nc.scalar.activation(sin32[:, csl], a_s, AF.Sin)
                nc.scalar.activation(nsin32[:, csl], a_s, AF.Sin, scale=-1.0)
                shifted = rope_p.tile([32, SB], F32, tag="rr_sh")
                nc.vector.tensor_scalar_add(shifted, angv, float(np.pi / 2))
                a_c = reduce_to_pi(shifted)
                nc.scalar.activation(cos32[:, csl], a_c, AF.Sin)
            # cosD [128, S] bf16 = cos32 x4 ; sinPM [128, S] = [-sin x2 ; +sin x2]
            cosD = consts.tile([128, S], BF, tag="cosD")
            sinPM = consts.tile([128, S], BF, tag="sinPM")
            for r in range(4):
                nc.sync.dma_start(out=cosD[32 * r : 32 * (r + 1), :], in_=cos32)
            nc.sync.dma_start(out=sinPM[0:32, :], in_=nsin32)
            nc.sync.dma_start(out=sinPM[32:64, :], in_=nsin32)
            nc.sync.dma_start(out=sinPM[64:96, :], in_=sin32)
            nc.sync.dma_start(out=sinPM[96:128, :], in_=sin32)

            # ones row for the K=1 linv broadcast matmul (partition 64)
            ones_sel = consts.tile([65, 64], BF, tag="ones_sel")
            nc.vector.memset(ones_sel, 1.0)

            # ---------------- causal masks ----------------
            # mask_j [128, 1024] bf16 : keep (f%512) - p - 128*j >= 0
            masks = []
            for j in range(4):
                mj = consts.tile([128, 2, SB], BF, tag=f"mask{j}")
                nc.gpsimd.memset(mj, 1.0)
                nc.gpsimd.affine_select(
                    out=mj,
                    in_=mj,
                    compare_op=mybir.AluOpType.is_ge,
                    fill=0.0,
                    base=-128 * j,
                    pattern=[[0, 2], [1, SB]],
                    channel_multiplier=-1,
                )
                masks.append(mj)

            # ---------------- main loop over batches ----------------
            pending = None
            for b in [bb for _ in range(reps) for bb in range(B)]:
                # x^T tiles for this batch
                xts = []
                for mt in range(8):
                    xt_t = xt_p.tile([128, S], BF, tag="xt")
                    nc.sync.dma_start(
                        out=xt_t, in_=xt_d[b, 128 * mt : 128 * (mt + 1), :]
                    )
                    xts.append(xt_t)

                # ---- Q^T / K^T projections + RoPE ----
                qr = qk_p.tile([128, S], BF, tag="qr")
                kr = qk_p.tile([128, S], BF, tag="kr")
                for (w_sb, dst) in ((wq_sb, qr), (wk_sb, kr)):
                    for sb_i in range(NSB):
                        ssl = slice(sb_i * SB, (sb_i + 1) * SB)
                        tps = ps2_p.tile([128, SB], F32, tag="proj")
                        for mt in range(8):
                            nc.tensor.matmul(
                                tps,
                                lhsT=w_sb[:, mt, :],
                                rhs=xts[mt][:, ssl],
                                start=(mt == 0),
                                stop=(mt == 7),
                            )
                        tsb = rope_p.tile([128, SB], BF, tag="tsb")
                        nc.any.tensor_copy(tsb, tps)
                        tswap = rope_p.tile([128, SB], BF, tag="tswap")
                        nc.scalar.dma_start(out=tswap[0:64, :], in_=tsb[64:128, :])
                        nc.scalar.dma_start(out=tswap[64:128, :], in_=tsb[0:64, :])
                        tcos = rope_p.tile([128, SB], BF, tag="tcos")
                        nc.vector.tensor_mul(tcos, tsb, cosD[:, ssl])
                        tsin = rope_p.tile([128, SB], BF, tag="tsin")
                        nc.vector.tensor_mul(tsin, tswap, sinPM[:, ssl])
                        trot = rope_p.tile([128, SB], BF, tag="trot")
                        nc.vector.tensor_add(trot, tcos, tsin)
                        # L1 [h0t1|h1t1|h0t2|h1t2] -> L2 [h0t1|h0t2|h1t1|h1t2]
                        # L1 [h0r1|h1r1|h0r2|h1r2] -> L2 [h0r1|h0r2|h1r1|h1r2]
                        for (dpo, spo) in ((0, 0), (32, 64), (64, 32), (96, 96)):
                            nc.scalar.dma_start(
                                out=dst[dpo : dpo + 32, ssl],
                                in_=trot[spo : spo + 32, :],
                            )

                # ---- V projection (natural [s, d] layout + ones cols) ----
                v_sb = v_p.tile([128, NKT, 130], BF, tag="v")
                nc.vector.memset(v_sb, 1.0)
                for kt in range(NKT):
                    vps = ps2_p.tile([128, SB], F32, tag="proj")
                    for mt in range(8):
                        nc.tensor.matmul(
                            vps[:, 0:128],
                            lhsT=xts[mt][:, 128 * kt : 128 * (kt + 1)],
                            rhs=wv_sb[:, mt, :],
                            start=(mt == 0),
                            stop=(mt == 7),
                        )
                    nc.vector.tensor_copy(v_sb[:, kt, 0:64], vps[:, 0:64])
                    nc.vector.tensor_copy(v_sb[:, kt, 65:129], vps[:, 64:128])

                # ---- attention per q-block ----
                def norm_and_outproj(b, qb, y0, y1):
                    # softmax denominators -> 1/l (bf16), K=1 matmul broadcast
                    linv0 = lin_p.tile([65, SB], BF, tag="linv0")
                    linv1 = lin_p.tile([65, SB], BF, tag="linv1")
                    with nc.allow_low_precision("softmax 1/l in bf16"):
                        nc.vector.reciprocal(linv0[64:65, :], y0[64:65, :])
                        nc.vector.reciprocal(linv1[64:65, :], y1[64:65, :])
                    ysb = ysb_p.tile([128, SB], BF, tag="ysb")
                    ytmp = ysb_p.tile([64, SB], BF, tag="ytmp")
                    for linv, ydata, yout in (
                        (linv0, y0, ysb[0:64, :]),
                        (linv1, y1, ytmp),
                    ):
                        if CFG.get("LB_IN_Y"):
                            lb_ps = y_p.tile([128, SB], F32, tag="y")
                        else:
                            lb_ps = ps_p.tile([128, SB], F32, tag="ps1")
                        nc.tensor.matmul(
                            lb_ps[0:64, :],
                            lhsT=ones_sel[64:65, :],
                            rhs=linv[64:65, :],
                            start=True,
                            stop=True,
                        )
                        lb_sb = lin_p.tile([64, SB], F32, tag="lb")
                        nc.any.tensor_copy(lb_sb, lb_ps[0:64, :])
                        nc.vector.tensor_mul(yout, ydata[0:64, :], lb_sb)
                    nc.scalar.dma_start(out=ysb[64:128, :], in_=ytmp)

                    # ---- output projection for this q-block ----
                    for jj in range(4):
                        s0 = qb * SB + 128 * jj
                        for mc in range(2):
                            msl = slice(512 * mc, 512 * (mc + 1))
                            ops = ps2_p.tile([128, SB], F32, tag="proj")
                            nc.tensor.matmul(
                                ops,
                                lhsT=ysb[:, 128 * jj : 128 * (jj + 1)],
                                rhs=wo_sb[:, msl],
                                start=True,
                                stop=True,
                            )
                            osb = outs_p.tile([128, 512], F32, tag="osb")
                            nc.any.tensor_copy(osb, ops)
                            nc.sync.dma_start(
                                out=out_d[b, s0 : s0 + 128, msl], in_=osb
                            )

                for qb in range(NSB):
                    qsl = slice(qb * SB, (qb + 1) * SB)
                    nkb = 4 * (qb + 1)
                    y0 = y_p.tile([128, SB], F32, tag="y")
                    y1 = y_p.tile([128, SB], F32, tag="y")
                    for kb in range(nkb):
                        ksl = slice(128 * kb, 128 * (kb + 1))
                        if CFG["SMODE"] == "pair":
                            s_t = s_p.tile([128, 2, SB], F32, tag="s")
                            s0v, s1v = s_t[:, 0, :], s_t[:, 1, :]
                            exp_srcs = [(s_t, None)]
                        else:
                            s0v = s_p.tile([128, SB], F32, tag="s")
                            s1v = s_p.tile([128, SB], F32, tag="s")
                            exp_srcs = [(s0v, 0), (s1v, 1)]
                        nc.tensor.matmul(
                            s0v, lhsT=kr[0:64, ksl], rhs=qr[0:64, qsl],
                            start=True, stop=True,
                        )
                        nc.tensor.matmul(
                            s1v, lhsT=kr[64:128, ksl], rhs=qr[64:128, qsl],
                            start=True, stop=True,
                        )
                        p_t = p_p.tile([128, 2, SB], BF, tag="p")
                        for src, half in exp_srcs:
                            dst_ap = p_t if half is None else p_t[:, half, :]
                            nc.scalar.activation(dst_ap, src, AF.Exp, scale=0.125)
                        j = kb - 4 * qb
                        if j >= 0:
                            nc.vector.tensor_mul(p_t, p_t, masks[j])
                        nc.tensor.matmul(
                            y0[0:65, :],
                            lhsT=v_sb[:, kb, 0:65],
                            rhs=p_t[:, 0, :],
                            start=(kb == 0),
                            stop=(kb == nkb - 1),
                        )
                        nc.tensor.matmul(
                            y1[0:65, :],
                            lhsT=v_sb[:, kb, 65:130],
                            rhs=p_t[:, 1, :],
                            start=(kb == 0),
                            stop=(kb == nkb - 1),
                        )
                    if CFG["DEFER"]:
                        if pending is not None:
                            norm_and_outproj(*pending)
                        pending = (b, qb, y0, y1)
                    else:
                        norm_and_outproj(b, qb, y0, y1)

            if pending is not None:
                norm_and_outproj(*pending)

    nc.compile()
    return nc


_NC_CACHE = {}


def get_nc(reps=1):
    if reps not in _NC_CACHE:
        _NC_CACHE[reps] = _build_nc(reps)
    return _NC_CACHE[reps]


def make_in_maps(x, token_positions, Wq, Wk, Wv, Wo):
    x = np.asarray(x, dtype=np.float32)
    Wq, Wk, Wv, Wo = (np.asarray(w, dtype=np.float32) for w in (Wq, Wk, Wv, Wo))
    pos = np.ascontiguousarray(
        np.asarray(token_positions, dtype=np.int32).reshape(1, S)
    )
    xt = np.ascontiguousarray(x.transpose(0, 2, 1)).astype(BF16)
    in_maps = []
    for c in range(NCORES):
        h0, h1 = 2 * c, 2 * c + 1
        rows = np.concatenate(
            [
                64 * h0 + np.arange(0, 64, 2),
                64 * h1 + np.arange(0, 64, 2),
                64 * h0 + np.arange(1, 64, 2),
                64 * h1 + np.arange(1, 64, 2),
            ]
        )
        in_maps.append(
            {
                "xt": xt,
                "wqt": np.ascontiguousarray(Wq[rows, :].T).astype(BF16),
                "wkt": np.ascontiguousarray(Wk[rows, :].T).astype(BF16),
                "wvt": np.ascontiguousarray(
                    Wv[128 * c : 128 * (c + 1), :].T
                ).astype(BF16),
                "wot": np.ascontiguousarray(
                    Wo[:, 128 * c : 128 * (c + 1)].T
                ).astype(BF16),
                "pos": pos,
            }
        )
    return in_maps


def kernel(x, token_positions, Wq, Wk, Wv, Wo):
    from concourse.bass_utils import run_bass_kernel_spmd

    nc = get_nc()
    in_maps = make_in_maps(x, token_positions, Wq, Wk, Wv, Wo)
    res = run_bass_kernel_spmd(nc, in_maps, core_ids=list(range(NCORES)))
    out = np.zeros((B, S, DM), np.float32)
    for r in res.results:
        out += r["outp"]
    return out



# revision 2
# speedup vs baseline: 1.4172x; 1.4172x over previous
"""Multi-head self-attention with RoPE on 8 Trainium2 NeuronCores — v2.

Sharding: tensor-parallel over heads (2 heads/core, all 4 batches).
Key changes vs v1 baseline:
  - RoPE trig tables computed on host (kills ~25us startup serial chain).
  - Head-contiguous W row order [h0e,h0o,h1e,h1o]; rotate-half swap done by
    4 partition-shifted tensor_adds (no intra-rope DMAs, no reorder DMAs).
  - Attention per q-block split into phase1 (scores->exp->P in SBUF) and
    phase2 (AV with P as lhsT, Y in [q,d] layout at full PE util).
  - Software pipelining: next batch's projection chunks are woven into the
    current batch's attention stream so the PE fills exp-bound stalls.
  - Causal skipping at 128-col granularity in scores/exp/AV.
  - Softmax denominators via ones-column of V; per-partition tensor_scalar
    normalization; PE transposes of Y feed the out-projection.
  - Act engine runs exp only (head-paired); copies spread over DVE/Pool.
  - bf16 partial outputs (host sums in f32).
"""

import numpy as np
import ml_dtypes

BF16 = ml_dtypes.bfloat16

B, S, DM = 4, 2048, 1024
H, DH = 16, 64
NCORES = 8
HPC = H // NCORES  # 2
DL = HPC * DH  # 128
SB = 512
NSB = S // SB  # 4
NKT = S // 128  # 16

CFG = {
    "XT_BUFS": 16,
    "QK_BUFS": 4,
    "ROPE_BUFS": 4,
    "V_BUFS": 3,
    "P_BUFS": 28,
    "YSN_BUFS": 10,
    "YST_BUFS": 6,
    "OSB_BUFS": 3,
    "LIN_BUFS": 4,
    "PROJ_PS": 1,
    "S_PS": 2,
    "Y_PS": 2,
    "TP_PS": 1,
}


def _weave(a, b, bias=1.0):
    """Proportionally interleave two unit lists (a leads).

    bias > 1 front-loads stream b (b finishes when a is at 1/bias of
    its length)."""
    out = []
    ia = ib = 0
    na, nb = len(a), len(b)
    while ia < na or ib < nb:
        fa = ia / na * bias if na else 1e9
        fb = ib / nb if nb else 1e9
        if ia < na and (fa <= fb or ib >= nb):
            out.append(a[ia])
            ia += 1
        else:
            out.append(b[ib])
            ib += 1
    return out


def _build_nc(reps=1, nbatch=B):
    import concourse.bass as bass
    import concourse.tile as tile
    import concourse.mybir as mybir
    from concourse import bacc

    dt = mybir.dt
    F32 = dt.float32
    BF = dt.bfloat16
    AF = mybir.ActivationFunctionType

    nc = bacc.Bacc("TRN2", target_bir_lowering=False, debug=False)

    xt_d = nc.dram_tensor("xt", [nbatch, DM, S], BF, kind="ExternalInput").ap()
    wq_d = nc.dram_tensor("wqt", [DM, DL], BF, kind="ExternalInput").ap()
    wk_d = nc.dram_tensor("wkt", [DM, DL], BF, kind="ExternalInput").ap()
    wv_d = nc.dram_tensor("wvt", [DM, DL], BF, kind="ExternalInput").ap()
    wo_d = nc.dram_tensor("wot", [DL, DM], BF, kind="ExternalInput").ap()
    cos_d = nc.dram_tensor("cosd", [128, S], BF, kind="ExternalInput").ap()
    sin_d = nc.dram_tensor("sintab", [128, S], BF, kind="ExternalInput").ap()
    id_d = nc.dram_tensor("ident", [128, 128], BF, kind="ExternalInput").ap()
    dm_d = nc.dram_tensor("dmask", [128, 128], BF, kind="ExternalInput").ap()
    out_d = nc.dram_tensor("outp", [nbatch, S, DM], BF, kind="ExternalOutput").ap()

    with tile.TileContext(nc) as tc:
        import contextlib

        ctx = contextlib.ExitStack()
        with ctx:
            consts = ctx.enter_context(tc.tile_pool(name="consts", bufs=1))
            xt_p = ctx.enter_context(tc.tile_pool(name="xt", bufs=CFG["XT_BUFS"]))
            qk_p = ctx.enter_context(tc.tile_pool(name="qk", bufs=CFG["QK_BUFS"]))
            rope_p = ctx.enter_context(tc.tile_pool(name="rope", bufs=CFG["ROPE_BUFS"]))
            v_p = ctx.enter_context(tc.tile_pool(name="v", bufs=CFG["V_BUFS"]))
            p_p = ctx.enter_context(tc.tile_pool(name="p", bufs=CFG["P_BUFS"]))
            ysn_p = ctx.enter_context(tc.tile_pool(name="ysn", bufs=CFG["YSN_BUFS"]))
            yst_p = ctx.enter_context(tc.tile_pool(name="yst", bufs=CFG["YST_BUFS"]))
            osb_p = ctx.enter_context(tc.tile_pool(name="osb", bufs=CFG["OSB_BUFS"]))
            lin_p = ctx.enter_context(tc.tile_pool(name="lin", bufs=CFG["LIN_BUFS"]))
            ps_proj = ctx.enter_context(
                tc.tile_pool(name="psproj", bufs=CFG["PROJ_PS"], space="PSUM")
            )
            s_p = ctx.enter_context(
                tc.tile_pool(name="s", bufs=CFG["S_PS"], space="PSUM")
            )
            y_p = ctx.enter_context(
                tc.tile_pool(name="y", bufs=CFG["Y_PS"], space="PSUM")
            )
            tp_p = ctx.enter_context(
                tc.tile_pool(name="tp", bufs=CFG["TP_PS"], space="PSUM")
            )

            # ---------------- consts -> SBUF ----------------
            wq_sb = consts.tile([128, 8, DL], BF, tag="wq")
            wk_sb = consts.tile([128, 8, DL], BF, tag="wk")
            wv_sb = consts.tile([128, 8, DL], BF, tag="wv")
            wo_sb = consts.tile([128, DM], BF, tag="wo")
            cosd = consts.tile([128, S], BF, tag="cosd")
            sintab = consts.tile([128, S], BF, tag="sintab")
            ident = consts.tile([128, 128], BF, tag="ident")
            dmask = consts.tile([128, 128], BF, tag="dmask")
            nc.sync.dma_start(out=wq_sb, in_=wq_d.rearrange("(t p) d -> p t d", p=128))
            nc.sync.dma_start(out=wk_sb, in_=wk_d.rearrange("(t p) d -> p t d", p=128))
            nc.sync.dma_start(out=wv_sb, in_=wv_d.rearrange("(t p) d -> p t d", p=128))
            nc.sync.dma_start(out=cosd, in_=cos_d)
            nc.sync.dma_start(out=sintab, in_=sin_d)
            nc.sync.dma_start(out=wo_sb, in_=wo_d)
            nc.sync.dma_start(out=ident, in_=id_d)
            nc.sync.dma_start(out=dmask, in_=dm_d)

            batches = [bb for _ in range(reps) for bb in range(nbatch)]
            NBAT = len(batches)

            xt_tiles = {}

            def fetch_xt(bi):
                if bi >= NBAT or bi in xt_tiles:
                    return
                b = batches[bi]
                tiles = []
                for mt in range(8):
                    t = xt_p.tile([128, S], BF, tag="xt")
                    nc.scalar.dma_start(
                        out=t, in_=xt_d[b, 128 * mt : 128 * (mt + 1), :]
                    )
                    tiles.append(t)
                xt_tiles[bi] = tiles

            proj_state = {}

            def make_proj_units(bi):
                """Projection + rope emission units for batch index bi."""
                b = batches[bi]
                xts = xt_tiles.pop(bi)
                qr = qk_p.tile([128, S], BF, tag="qr")
                kr = qk_p.tile([128, S], BF, tag="kr")
                v_sb = v_p.tile([128, NKT, 130], BF, tag="v")
                proj_state[bi] = (qr, kr, v_sb)
                units = []

                rope_tmp = {}

                def rope_a(w_sb, key, sb_i):
                    ssl = slice(sb_i * SB, (sb_i + 1) * SB)
                    tps = ps_proj.tile([128, SB], F32, tag="proj")
                    for mt in range(8):
                        nc.tensor.matmul(
                            tps,
                            lhsT=w_sb[:, mt, :],
                            rhs=xts[mt][:, ssl],
                            start=(mt == 0),
                            stop=(mt == 7),
                        )
                    tsb = rope_p.tile([128, SB], BF, tag="tsb")
                    nc.vector.tensor_copy(tsb, tps)
                    tswap = rope_p.tile([128, SB], BF, tag="tswap")
                    for (dpo, spo) in ((0, 32), (32, 0), (64, 96), (96, 64)):
                        nc.sync.dma_start(
                            out=tswap[dpo : dpo + 32, :],
                            in_=tsb[spo : spo + 32, :],
                        )
                    rope_tmp[(key, sb_i)] = (tsb, tswap)

                def rope_b(dst, key, sb_i):
                    ssl = slice(sb_i * SB, (sb_i + 1) * SB)
                    tsb, tswap = rope_tmp.pop((key, sb_i))
                    tcos = rope_p.tile([128, SB], BF, tag="tcos")
                    nc.vector.tensor_mul(tcos, tsb, cosd[:, ssl])
                    tsin = rope_p.tile([128, SB], BF, tag="tsin")
                    nc.vector.tensor_mul(tsin, tswap, sintab[:, ssl])
                    nc.vector.tensor_add(dst[:, ssl], tcos, tsin)

                for (w_sb, dst, key) in ((wq_sb, qr, "q"), (wk_sb, kr, "k")):
                    for sb_i in range(NSB):
                        units.append(
                            lambda w_sb=w_sb, key=key, sb_i=sb_i: rope_a(
                                w_sb, key, sb_i
                            )
                        )
                        units.append(
                            lambda dst=dst, key=key, sb_i=sb_i: rope_b(
                                dst, key, sb_i
                            )
                        )

                def v_ones():
                    nc.vector.memset(v_sb[:, :, 64:65], 1.0)
                    nc.vector.memset(v_sb[:, :, 129:130], 1.0)

                units.append(v_ones)

                def v_chunk(kt):
                    vps = ps_proj.tile([128, SB], F32, tag="proj")
                    for mt in range(8):
                        nc.tensor.matmul(
                            vps[:, 0:128],
                            lhsT=xts[mt][:, 128 * kt : 128 * (kt + 1)],
                            rhs=wv_sb[:, mt, :],
                            start=(mt == 0),
                            stop=(mt == 7),
                        )
                    nc.vector.tensor_copy(v_sb[:, kt, 0:64], vps[:, 0:64])
                    nc.vector.tensor_copy(v_sb[:, kt, 65:129], vps[:, 64:128])

                for kt in range(NKT):
                    units.append(lambda kt=kt: v_chunk(kt))
                return units

            pending = [None]  # [(b, qb, p_list, y_pair, v_sb)]

            def av_unit(st, h, qc):
                b, qb, p_list, y_pair, v_sb = st[:5]
                kmax = 4 * qb + qc + 1
                y_t = y_pair[h]
                for kb in range(kmax):
                    nc.tensor.matmul(
                        y_t[:, qc, 0:65],
                        lhsT=p_list[kb][:, h, 128 * qc : 128 * (qc + 1)],
                        rhs=v_sb[:, kb, 65 * h : 65 * h + 65],
                        start=(kb == 0),
                        stop=(kb == kmax - 1),
                    )

            def finish_norm(st):
                b, qb, p_list, y_pair, v_sb = st
                fs = {"ysn": [], "ysbT": []}
                st.append(fs)
                for h in range(2):
                    y_t = y_pair[h]
                    linv = lin_p.tile([128, 4], F32, tag="linv")
                    nc.vector.reciprocal(linv, y_t[:, :, 64:65])
                    for qc in range(4):
                        ysn = ysn_p.tile([128, 64], BF, tag="ysn")
                        nc.vector.tensor_scalar(
                            ysn,
                            y_t[:, qc, 0:64],
                            linv[:, qc : qc + 1],
                            None,
                            op0=mybir.AluOpType.mult,
                        )
                        fs["ysn"].append(ysn)

            def finish_transpose(st):
                fs = st[-1]
                ysbT = []
                for _qc in range(4):
                    yt_t = yst_p.tile([128, 128], BF, tag="yst")
                    ysbT.append(yt_t)
                fs["ysbT"] = ysbT
                tp_all = tp_p.tile([128, 8, 128], BF, tag="tp")
                for h in range(2):
                    for qc in range(4):
                        ysn = fs["ysn"][4 * h + qc]
                        slot = tp_all[64 * h : 64 * h + 64, 2 * qc + h, :]
                        nc.tensor.transpose(
                            slot, ysn, ident, tile_position=(0, 64 * h)
                        )
                        nc.vector.tensor_copy(
                            ysbT[qc][64 * h : 64 * h + 64, :], slot
                        )

            def finish_outproj(st, qcs):
                b, qb = st[0], st[1]
                fs = st[-1]
                ysbT = fs["ysbT"]
                for qc in qcs:
                    s0 = qb * SB + 128 * qc
                    osb = osb_p.tile([128, DM], BF, tag="osb")
                    for mc in range(2):
                        msl = slice(512 * mc, 512 * (mc + 1))
                        ops = ps_proj.tile([128, SB], F32, tag="proj")
                        nc.tensor.matmul(
                            ops, lhsT=ysbT[qc], rhs=wo_sb[:, msl],
                            start=True, stop=True,
                        )
                        if mc == 0:
                            nc.scalar.copy(osb[:, msl], ops)
                        else:
                            nc.vector.tensor_copy(osb[:, msl], ops)
                    nc.sync.dma_start(out=out_d[b, s0 : s0 + 128, :], in_=osb)

            def make_attn_units(bi):
                b = batches[bi]
                qr, kr, v_sb = proj_state.pop(bi)
                units = []
                deferred_mask = [None]

                def flush_mask():
                    if deferred_mask[0] is not None:
                        p_t, j = deferred_mask[0]
                        deferred_mask[0] = None
                        for h in range(2):
                            nc.gpsimd.tensor_mul(
                                p_t[:, h, 128 * j : 128 * (j + 1)],
                                p_t[:, h, 128 * j : 128 * (j + 1)],
                                dmask,
                            )

                for qb in range(NSB):
                    nkb = 4 * (qb + 1)
                    p_list = []

                    def ph1(qb, kb, p_list):
                        j = kb - 4 * qb
                        qlo = 128 * j if j > 0 else 0
                        qsl = slice(qb * SB + qlo, (qb + 1) * SB)
                        ksl = slice(128 * kb, 128 * (kb + 1))
                        s_t = s_p.tile([128, 2, SB], F32, tag="s")
                        for h in range(2):
                            nc.tensor.matmul(
                                s_t[:, h, qlo:SB],
                                lhsT=kr[64 * h : 64 * h + 64, ksl],
                                rhs=qr[64 * h : 64 * h + 64, qsl],
                                start=True,
                                stop=True,
                            )
                        p_t = p_p.tile([128, 2, SB], BF, tag="p")
                        nc.scalar.activation(
                            p_t[:, :, qlo:SB], s_t[:, :, qlo:SB], AF.Exp,
                            scale=0.125,
                        )
                        flush_mask()
                        if j >= 0:
                            deferred_mask[0] = (p_t, j)
                        p_list.append(p_t)

                    ph1_units = [
                        (lambda qb=qb, kb=kb, p_list=p_list: ph1(qb, kb, p_list))
                        for kb in range(nkb)
                    ]
                    # pending[0] is resolved at EXECUTION time (set by the
                    # previous q-block's set_pending unit), not build time.
                    av_units = [
                        (
                            lambda h=h, qc=qc: (
                                av_unit(pending[0], h, qc)
                                if pending[0] is not None
                                else None
                            )
                        )
                        for qc in range(4)
                        for h in range(2)
                    ]
                    for fu in (
                        finish_norm,
                        finish_transpose,
                        lambda st: finish_outproj(st, (0, 1)),
                        lambda st: finish_outproj(st, (2, 3)),
                    ):
                        av_units.append(
                            lambda fu=fu: (
                                fu(pending[0]) if pending[0] is not None else None
                            )
                        )
                    units.extend(_weave(ph1_units, av_units))

                    def set_pending(qb=qb, p_list=p_list):
                        flush_mask()
                        y0 = y_p.tile([128, 4, 65], F32, tag="y")
                        y1 = y_p.tile([128, 4, 65], F32, tag="y")
                        pending[0] = [b, qb, p_list, [y0, y1], v_sb]

                    units.append(set_pending)
                return units

            # ---------------- main schedule ----------------
            fetch_xt(0)
            fetch_xt(1)
            for u in make_proj_units(0):
                u()
            for bi in range(NBAT):
                fetch_xt(bi + 2)
                proj_units = make_proj_units(bi + 1) if bi + 1 < NBAT else []
                attn_units = make_attn_units(bi)
                for u in _weave(attn_units, proj_units, bias=1.35):
                    u()
            st = pending[0]
            for qc in range(4):
                for h in range(2):
                    av_unit(st, h, qc)
            finish_norm(st)
            finish_transpose(st)
            finish_outproj(st, (0, 1, 2, 3))

    nc.compile()
    return nc


_NC_CACHE = {}


def get_nc(reps=1, nbatch=B):
    key = (reps, nbatch)
    if key not in _NC_CACHE:
        _NC_CACHE[key] = _build_nc(reps, nbatch)
    return _NC_CACHE[key]


def _rows_for_core(c):
    h0, h1 = 2 * c, 2 * c + 1
    return np.concatenate(
        [
            64 * h0 + np.arange(0, 64, 2),
            64 * h0 + np.arange(1, 64, 2),
            64 * h1 + np.arange(0, 64, 2),
            64 * h1 + np.arange(1, 64, 2),
        ]
    )


def _trig_tables(token_positions):
    pos = np.asarray(token_positions, dtype=np.float64).reshape(S)
    invf = 10000.0 ** (-np.arange(0, DH, 2, dtype=np.float64) / DH)  # [32]
    ang = pos[None, :] * invf[:, None]  # [32, S]
    cos32 = np.cos(ang)
    sin32 = np.sin(ang)
    cosd = np.tile(cos32, (4, 1)).astype(BF16)  # [128, S]
    sintab = np.concatenate([-sin32, sin32, -sin32, sin32], axis=0).astype(BF16)
    return cosd, sintab


def make_in_maps(x, token_positions, Wq, Wk, Wv, Wo, nbatch=B):
    x = np.asarray(x, dtype=np.float32)
    Wq, Wk, Wv, Wo = (np.asarray(w, dtype=np.float32) for w in (Wq, Wk, Wv, Wo))
    xt = np.ascontiguousarray(x[:nbatch].transpose(0, 2, 1)).astype(BF16)
    cosd, sintab = _trig_tables(token_positions)
    ident = np.eye(128, dtype=np.float32).astype(BF16)
    dmask = (np.arange(128)[None, :] >= np.arange(128)[:, None]).astype(BF16)
    in_maps = []
    for c in range(NCORES):
        rows = _rows_for_core(c)
        in_maps.append(
            {
                "xt": xt,
                "wqt": np.ascontiguousarray(Wq[rows, :].T).astype(BF16),
                "wkt": np.ascontiguousarray(Wk[rows, :].T).astype(BF16),
                "wvt": np.ascontiguousarray(
                    Wv[128 * c : 128 * (c + 1), :].T
                ).astype(BF16),
                "wot": np.ascontiguousarray(
                    Wo[:, 128 * c : 128 * (c + 1)].T
                ).astype(BF16),
                "cosd": cosd,
                "sintab": sintab,
                "ident": ident,
                "dmask": dmask,
            }
        )
    return in_maps


def kernel(x, token_positions, Wq, Wk, Wv, Wo):
    from concourse.bass_utils import run_bass_kernel_spmd

    nc = get_nc()
    in_maps = make_in_maps(x, token_positions, Wq, Wk, Wv, Wo)
    res = run_bass_kernel_spmd(nc, in_maps, core_ids=list(range(NCORES)))
    out = np.zeros((B, S, DM), np.float32)
    for r in res.results:
        out += r["outp"].astype(np.float32)
    return out
